# revision 6
# baseline (speedup 1.0000x reference)
"""Trainium2 Bass kernel for nn_MilliesRNN (B=256, T=512, L=128, O=64, H=576).

Strategy:
- Data-parallel over batch: 8 cores x 32 sequences each; all weights replicated.
- On each core the recurrent state is kept transposed ([hidden, batch] =
  [576->5 chunks of 128, 32]) so every matmul uses the weights as the
  stationary operand and the state as the 32-wide moving operand; states stay
  SBUF/PSUM-resident for all 512 steps.
- Algebraic refactor (validated to 2.5e-07 vs reference in fp32):
    z_v  = W1^T h_v + i2h^T x_t + bias_v (+ prev out_t on dims 512:576)
    h_v  = relu(tanh(z_v))
    p    = Mv^T h_v + thal_m^T out_m + bias_p + hold_t * holdw_p
    out_t= relu(tanh(p))           with Mv = (thal_w[:, :64] @ h2o_w)^T
    z_m  = W2^T h_m + Wtm^T out_t + bias_m + hold_t * holdw_m
    h_m  = relu(tanh(z_m))
    out_m= Mo^T h_m + h2od_b       (also the per-step output)
  Biases/hold enter as K=1/K=2 matmuls against a host-precomputed
  [ones; hold] stream.
- Compute dtype bf16 (weights + states; PSUM accumulation fp32, output fp32):
  measured 2.6e-03 rel L2 end-to-end in a bit-accurate numpy simulation.
"""

import numpy as np

import concourse.bass as bass
import concourse.mybir as mybir
import concourse.tile as tile
from concourse.bass_utils import run_bass_kernel_spmd
from concourse.vector_clock import ScopedClock

# ----------------------------------------------------------------------------
# Workarounds: this walrus build only supports ONE sync-wait per instruction.
# ----------------------------------------------------------------------------
_MAXW = 1


def _patched_drain_and_barrier(self, tick_clock, wait_clock):
    nc = self.nc
    drain_inst = nc.sync.drain()
    wait_clock.add_sem_waits(
        drain_inst.ins, ScopedClock({None: tick_clock.global_clock})
    )
    waits = list(drain_inst.ins.sync_info.on_wait)
    if len(waits) > _MAXW:
        drain_inst.ins.sync_info = mybir.SyncInfo(
            on_wait=waits[:_MAXW], on_update=[]
        )
        rest = waits[_MAXW:]
        for i in range(0, len(rest), _MAXW):
            nop = nc.sync.nop(nofuse=True)
            nop.ins.sync_info = mybir.SyncInfo(
                on_wait=rest[i : i + _MAXW], on_update=[]
            )
    nc.all_engine_barrier()
    assert self.sems is not None
    popped = nc._tile_sem_poison_stack.pop()
    assert popped is self._sem_poison
    nc.clear_and_free_semaphores(list(self.sems.allocated().values()))
    nc.all_engine_barrier()


tile.TileContext._drain_and_barrier = _patched_drain_and_barrier

_wfix_ctr = [0]


def _split_waits(nc, maxw=_MAXW):
    """Move excess sync-waits onto preceding same-engine nops."""
    n_split = 0
    for f in nc.m.functions:
        for b in f.blocks:
            lst = b.instructions
            i = 0
            while i < len(lst):
                inst = lst[i]
                si = getattr(inst, "sync_info", None)
                if si is not None:
                    waits = list(si.on_wait)
                    if len(waits) > maxw:
                        n_split += 1
                        inst.sync_info = mybir.SyncInfo(
                            on_wait=waits[:maxw], on_update=list(si.on_update)
                        )
                        rest = waits[maxw:]
                        for j in range(0, len(rest), maxw):
                            nop = mybir.InstNoOp(
                                name=f"WFIX-{_wfix_ctr[0]}", ins=[], outs=[]
                            )
                            _wfix_ctr[0] += 1
                            nop.engine = inst.engine
                            nop.sync_info = mybir.SyncInfo(
                                on_wait=rest[j : j + maxw], on_update=[]
                            )
                            lst.insert(i, nop)
                            i += 1
                i += 1
    return n_split


# ----------------------------------------------------------------------------
# Problem constants
# ----------------------------------------------------------------------------
B, T, L, O, H = 256, 512, 128, 64, 576
N_CORES = 8
BC = B // N_CORES  # 32 batch per core
NCH = 5  # hidden chunks: 128*4 + 64
CHW = [128, 128, 128, 128, 64]
F32 = mybir.dt.float32
AF = mybir.ActivationFunctionType
ALU = mybir.AluOpType


def build_nc(T_steps=T, cd=mybir.dt.bfloat16):
    """Build the per-core SPMD bass kernel (fully unrolled over T_steps)."""
    nc = bass.Bass()
    TB = T_steps * BC

    # DRAM I/O (per core). Host packs these exactly in SBUF layout.
    d_xT = nc.dram_tensor("xT", [L, TB], cd, kind="ExternalInput")
    d_oh = nc.dram_tensor("OH", [2, TB], cd, kind="ExternalInput")
    d_w1 = nc.dram_tensor("W1s", [128, NCH * H], cd, kind="ExternalInput")
    d_w2 = nc.dram_tensor("W2s", [128, NCH * H], cd, kind="ExternalInput")
    d_i2h = nc.dram_tensor("I2Hs", [128, 512], cd, kind="ExternalInput")
    d_mv = nc.dram_tensor("MVs", [128, NCH * O], cd, kind="ExternalInput")
    d_mo = nc.dram_tensor("MOs", [128, NCH * O], cd, kind="ExternalInput")
    d_wtm = nc.dram_tensor("WTMs", [O, H], cd, kind="ExternalInput")
    d_thm = nc.dram_tensor("THM", [O, O], cd, kind="ExternalInput")
    d_id = nc.dram_tensor("ID64", [O, O], cd, kind="ExternalInput")
    d_bv = nc.dram_tensor("BV", [1, H], cd, kind="ExternalInput")
    d_bm = nc.dram_tensor("BM2", [2, H], cd, kind="ExternalInput")
    d_bp = nc.dram_tensor("BP", [2, O], cd, kind="ExternalInput")
    d_bo = nc.dram_tensor("BO", [1, O], cd, kind="ExternalInput")
    d_h0v = nc.dram_tensor("H0V", [128, NCH * BC], cd, kind="ExternalInput")
    d_h0m = nc.dram_tensor("H0M", [128, NCH * BC], cd, kind="ExternalInput")
    d_out = nc.dram_tensor("OUT", [O, TB], F32, kind="ExternalOutput")

    frees = []
    with tile.TileContext(nc) as tc:

        def mk(name, shape, dtype):
            t, fr = tc.tile(shape, dtype, name=name)
            frees.append(fr)
            return t

        xTs = mk("xTs", [L, TB], cd)
        oh = mk("oh", [2, TB], cd)
        w1 = mk("w1", [128, NCH * H], cd)
        w2 = mk("w2", [128, NCH * H], cd)
        i2h = mk("i2h", [128, 512], cd)
        mv = mk("mv", [128, NCH * O], cd)
        mo = mk("mo", [128, NCH * O], cd)
        wtm = mk("wtm", [O, H], cd)
        thm = mk("thm", [O, O], cd)
        id64 = mk("id64", [O, O], cd)
        bv = mk("bv", [1, H], cd)
        bm = mk("bm", [2, H], cd)
        bp = mk("bp", [2, O], cd)
        bo = mk("bo", [1, O], cd)
        hv = mk("hv", [128, NCH * BC], cd)
        hm = mk("hm", [128, NCH * BC], cd)
        ot = mk("ot", [O, BC], cd)
        om = mk("om", [O, BC], cd)
        outb = mk("outb", [O, TB], F32)

        # --- load everything (xT split along T for early compute start) ---
        NXCH = 8
        xch = TB // NXCH
        for i in range(NXCH):
            nc.sync.dma_start(
                out=xTs[:, i * xch : (i + 1) * xch],
                in_=d_xT[:, i * xch : (i + 1) * xch],
            )
        nc.sync.dma_start(out=oh, in_=d_oh[:, :])
        nc.sync.dma_start(out=w1, in_=d_w1[:, :])
        nc.sync.dma_start(out=w2, in_=d_w2[:, :])
        nc.sync.dma_start(out=i2h, in_=d_i2h[:, :])
        nc.sync.dma_start(out=mv, in_=d_mv[:, :])
        nc.sync.dma_start(out=mo, in_=d_mo[:, :])
        nc.sync.dma_start(out=wtm, in_=d_wtm[:, :])
        nc.sync.dma_start(out=thm, in_=d_thm[:, :])
        nc.sync.dma_start(out=id64, in_=d_id[:, :])
        nc.sync.dma_start(out=bv, in_=d_bv[:, :])
        nc.sync.dma_start(out=bm, in_=d_bm[:, :])
        nc.sync.dma_start(out=bp, in_=d_bp[:, :])
        nc.sync.dma_start(out=bo, in_=d_bo[:, :])
        nc.sync.dma_start(out=hv, in_=d_h0v[:, :])
        nc.sync.dma_start(out=hm, in_=d_h0m[:, :])

        _pp_cm = tc.tile_pool(name="psum", bufs=2, space="PSUM")
        pp = _pp_cm.__enter__()

        def mm(out_ap, lhsT, rhs, start, stop):
            nc.tensor.matmul(
                out_ap, lhsT, rhs, start=start, stop=stop, skip_group_check=True
            )

        def emit_group(mms):
            n = len(mms)
            for i, (out_ap, lhsT, rhs) in enumerate(mms):
                mm(out_ap, lhsT, rhs, i == 0, i == n - 1)

        for t in range(T_steps):
            ts = slice(t * BC, (t + 1) * BC)
            # ---------------- z_v ----------------
            zv = pp.tile([128, NCH * BC], F32, tag="zv")
            g = []
            for m in range(NCH):
                m0, mw = m * 128, CHW[m]
                oc = slice(m * BC, m * BC + BC)
                for k in range(NCH):
                    kw = CHW[k]
                    g.append((zv[0:mw, oc],
                              w1[0:kw, k * H + m0 : k * H + m0 + mw],
                              hv[0:kw, k * BC : k * BC + BC]))
                if m < 4:
                    g.append((zv[0:mw, oc], i2h[0:128, m0 : m0 + mw],
                              xTs[:, ts]))
                g.append((zv[0:mw, oc], bv[0:1, m0 : m0 + mw], oh[0:1, ts]))
            if t > 0:
                g.append((zv[0:64, 4 * BC : 5 * BC], id64[0:64, 0:64],
                          ot[0:64, :]))
            emit_group(g)
            # ---------------- z_m part 1: W2 @ h_m + bias/hold ----------
            zm = pp.tile([128, NCH * BC], F32, tag="zm")
            zmfirst = True
            for m in range(NCH):
                m0, mw = m * 128, CHW[m]
                oc = slice(m * BC, m * BC + BC)
                for k in range(NCH):
                    kw = CHW[k]
                    mm(
                        zm[0:mw, oc],
                        w2[0:kw, k * H + m0 : k * H + m0 + mw],
                        hm[0:kw, k * BC : k * BC + BC],
                        zmfirst,
                        False,
                    )
                    zmfirst = False
                mm(zm[0:mw, oc], bm[0:2, m0 : m0 + mw], oh[0:2, ts], False,
                   False)
            # ---------------- h_v = relu(tanh(z_v)) ----------------
            nc.scalar.activation(hv[0:128, 0 : 4 * BC], zv[0:128, 0 : 4 * BC],
                                 AF.Tanh)
            nc.scalar.activation(hv[0:64, 4 * BC : 5 * BC],
                                 zv[0:64, 4 * BC : 5 * BC], AF.Tanh)
            nc.vector.tensor_scalar_max(hv[0:128, 0 : 4 * BC],
                                        hv[0:128, 0 : 4 * BC], 0.0)
            nc.vector.tensor_scalar_max(hv[0:64, 4 * BC : 5 * BC],
                                        hv[0:64, 4 * BC : 5 * BC], 0.0)
            # ---------------- p -> out_t ----------------
            pt = pp.tile([O, BC], F32, tag="pt")
            pfirst = True
            for k in range(NCH):
                kw = CHW[k]
                mm(pt[0:64, :], mv[0:kw, k * O : k * O + O],
                   hv[0:kw, k * BC : k * BC + BC], pfirst, False)
                pfirst = False
            if t > 0:
                mm(pt[0:64, :], thm[0:64, 0:64], om[0:64, :], False, False)
            mm(pt[0:64, :], bp[0:2, 0:64], oh[0:2, ts], False, True)
            nc.scalar.activation(ot[0:64, :], pt[0:64, :], AF.Tanh)
            nc.vector.tensor_scalar_max(ot[0:64, :], ot[0:64, :], 0.0)
            # ---------------- z_m part 2: Wtm @ out_t ----------------
            for m in range(NCH):
                m0, mw = m * 128, CHW[m]
                oc = slice(m * BC, m * BC + BC)
                mm(zm[0:mw, oc], wtm[0:64, m0 : m0 + mw], ot[0:64, :], False,
                   m == NCH - 1)
            # ---------------- h_m = relu(tanh(z_m)) ----------------
            nc.scalar.activation(hm[0:128, 0 : 4 * BC], zm[0:128, 0 : 4 * BC],
                                 AF.Tanh)
            nc.scalar.activation(hm[0:64, 4 * BC : 5 * BC],
                                 zm[0:64, 4 * BC : 5 * BC], AF.Tanh)
            nc.vector.tensor_scalar_max(hm[0:128, 0 : 4 * BC],
                                        hm[0:128, 0 : 4 * BC], 0.0)
            nc.vector.tensor_scalar_max(hm[0:64, 4 * BC : 5 * BC],
                                        hm[0:64, 4 * BC : 5 * BC], 0.0)
            # ---------------- out_m ----------------
            zo = pp.tile([O, BC], F32, tag="zo")
            ofirst = True
            for k in range(NCH):
                kw = CHW[k]
                mm(zo[0:64, :], mo[0:kw, k * O : k * O + O],
                   hm[0:kw, k * BC : k * BC + BC], ofirst, False)
                ofirst = False
            mm(zo[0:64, :], bo[0:1, 0:64], oh[0:1, ts], False, True)
            nc.scalar.activation(outb[0:64, ts], zo[0:64, :], AF.Copy)
            nc.vector.tensor_copy(om[0:64, :], zo[0:64, :])

        # final store
        NOCH = 8
        och = TB // NOCH
        for i in range(NOCH):
            nc.sync.dma_start(
                out=d_out[:, i * och : (i + 1) * och],
                in_=outb[:, i * och : (i + 1) * och],
            )
        _pp_cm.__exit__(None, None, None)
        for fr in reversed(frees):
            fr()

    _split_waits(nc)
    return nc


# ----------------------------------------------------------------------------
# Host-side packing
# ----------------------------------------------------------------------------
def _np_dt(cd):
    if cd == mybir.dt.bfloat16:
        import ml_dtypes

        return ml_dtypes.bfloat16
    return np.float32


def pack_inputs(inputs, T_steps=T, cd=mybir.dt.bfloat16):
    npdt = _np_dt(cd)
    d = inputs

    def C(x):
        return np.ascontiguousarray(x.astype(np.float32)).astype(npdt)

    # shared (replicated) weights
    w1f = np.zeros((128, NCH * H), np.float32)
    w2f = np.zeros((128, NCH * H), np.float32)
    W1T = d["h2h_w"].T.astype(np.float32)  # [576, 576]
    W2T = d["h2hd_w"].T.astype(np.float32)
    for k in range(NCH):
        kw = CHW[k]
        w1f[0:kw, k * H : (k + 1) * H] = W1T[k * 128 : k * 128 + kw, :]
        w2f[0:kw, k * H : (k + 1) * H] = W2T[k * 128 : k * 128 + kw, :]
    Mv = (d["thal_w"][:, :64].astype(np.float32)
          @ d["h2o_w"].astype(np.float32)).T  # [576, 64]
    MoT = d["h2od_w"].T.astype(np.float32)  # [576, 64]
    mvf = np.zeros((128, NCH * O), np.float32)
    mof = np.zeros((128, NCH * O), np.float32)
    for k in range(NCH):
        kw = CHW[k]
        mvf[0:kw, k * O : (k + 1) * O] = Mv[k * 128 : k * 128 + kw, :]
        mof[0:kw, k * O : (k + 1) * O] = MoT[k * 128 : k * 128 + kw, :]
    bias_v = d["h2h_b"].astype(np.float32).copy()
    bias_v[:512] += d["i2h_b"].astype(np.float32)
    bias_p = (d["thal_b"].astype(np.float32)
              + d["thal_w"][:, :64].astype(np.float32)
              @ d["h2o_b"].astype(np.float32))
    holdw_p = d["thal_w"][:, 64].astype(np.float32)
    bias_m = d["i2hd_b"].astype(np.float32) + d["h2hd_b"].astype(np.float32)
    holdw_m = d["i2hd_w"][:, 64].astype(np.float32)

    shared = {
        "W1s": C(w1f),
        "W2s": C(w2f),
        "I2Hs": C(d["i2h_w"].T),
        "MVs": C(mvf),
        "MOs": C(mof),
        "WTMs": C(d["i2hd_w"][:, :64].T),
        "THM": C(d["thal_w"][:, 65:].T),
        "ID64": C(np.eye(64, dtype=np.float32)),
        "BV": C(bias_v[None, :]),
        "BM2": C(np.stack([bias_m, holdw_m])),
        "BP": C(np.stack([bias_p, holdw_p])),
        "BO": C(d["h2od_b"][None, :]),
    }

    data = d["data"].astype(np.float32)
    in_maps = []
    for c in range(N_CORES):
        s = slice(c * BC, (c + 1) * BC)
        dc = data[s, :T_steps, :]  # [BC, T, L]
        xT = np.ascontiguousarray(dc.transpose(2, 1, 0)).reshape(L, -1)
        ohrow = np.ones((2, T_steps * BC), np.float32)
        ohrow[1] = np.ascontiguousarray(dc[:, :, 127].T).reshape(-1)
        h0v = np.zeros((128, NCH * BC), np.float32)
        h0m = np.zeros((128, NCH * BC), np.float32)
        hv0 = d["h0_v"][s].astype(np.float32).T  # [576, BC]
        hm0 = d["h0_m"][s].astype(np.float32).T
        for k in range(NCH):
            kw = CHW[k]
            h0v[0:kw, k * BC : (k + 1) * BC] = hv0[k * 128 : k * 128 + kw, :]
            h0m[0:kw, k * BC : (k + 1) * BC] = hm0[k * 128 : k * 128 + kw, :]
        m = dict(shared)
        m["xT"] = xT.astype(_np_dt(cd))
        m["OH"] = ohrow.astype(_np_dt(cd))
        m["H0V"] = h0v.astype(_np_dt(cd))
        m["H0M"] = h0m.astype(_np_dt(cd))
        in_maps.append(m)
    return in_maps


def run(inputs, T_steps=T, cd=mybir.dt.bfloat16, nc=None):
    if nc is None:
        nc = build_nc(T_steps, cd)
    in_maps = pack_inputs(inputs, T_steps, cd)
    res = run_bass_kernel_spmd(nc, in_maps, core_ids=list(range(N_CORES)))
    out = np.zeros((B, T_steps, O), np.float32)
    for c in range(N_CORES):
        o = res.results[c]["OUT"]  # [O, T*BC]
        out[c * BC : (c + 1) * BC] = (
            o.reshape(O, T_steps, BC).transpose(2, 1, 0)
        )
    return out


def kernel(**inputs):
    return run(inputs)


if __name__ == "__main__":
    pass


# revision 19
# speedup vs baseline: 1214.4417x; 1214.4417x over previous
"""Trainium2 Bass kernel for nn_MilliesRNN (B=256, T=512, L=128, O=64, H=576).

Strategy:
- Data-parallel over batch: 8 cores x 32 sequences each; all weights replicated.
- On each core the recurrent state is kept transposed ([hidden, batch] =
  [576->5 chunks of 128, 32]) so every matmul uses the weights as the
  stationary operand and the state as the 32-wide moving operand; states stay
  SBUF/PSUM-resident for all 512 steps.
- Algebraic refactor (validated to 2.5e-07 vs reference in fp32):
    z_v  = W1^T h_v + i2h^T x_t + bias_v (+ prev out_t on dims 512:576)
    h_v  = relu(tanh(z_v))
    p    = Mv^T h_v + thal_m^T out_m + bias_p + hold_t * holdw_p
    out_t= relu(tanh(p))           with Mv = (thal_w[:, :64] @ h2o_w)^T
    z_m  = W2^T h_m + Wtm^T out_t + bias_m + hold_t * holdw_m
    h_m  = relu(tanh(z_m))
    out_m= Mo^T h_m + h2od_b       (also the per-step output)
  Biases/hold enter as K=1/K=2 matmuls against a host-precomputed
  [ones; hold] stream.
- Compute dtype bf16 (weights + states; PSUM accumulation fp32, output fp32):
  measured 2.6e-03 rel L2 end-to-end in a bit-accurate numpy simulation.
"""

import numpy as np

import concourse.bass as bass
import concourse.mybir as mybir
import concourse.tile as tile
from concourse.bass_utils import run_bass_kernel_spmd
from concourse.vector_clock import ScopedClock

# ----------------------------------------------------------------------------
# Workarounds: this walrus build only supports ONE sync-wait per instruction.
# ----------------------------------------------------------------------------
_MAXW = 1


def _patched_drain_and_barrier(self, tick_clock, wait_clock):
    nc = self.nc
    drain_inst = nc.sync.drain()
    wait_clock.add_sem_waits(
        drain_inst.ins, ScopedClock({None: tick_clock.global_clock})
    )
    waits = list(drain_inst.ins.sync_info.on_wait)
    if len(waits) > _MAXW:
        drain_inst.ins.sync_info = mybir.SyncInfo(
            on_wait=waits[:_MAXW], on_update=[]
        )
        rest = waits[_MAXW:]
        for i in range(0, len(rest), _MAXW):
            nop = nc.sync.nop(nofuse=True)
            nop.ins.sync_info = mybir.SyncInfo(
                on_wait=rest[i : i + _MAXW], on_update=[]
            )
    nc.all_engine_barrier()
    assert self.sems is not None
    popped = nc._tile_sem_poison_stack.pop()
    assert popped is self._sem_poison
    nc.clear_and_free_semaphores(list(self.sems.allocated().values()))
    nc.all_engine_barrier()


tile.TileContext._drain_and_barrier = _patched_drain_and_barrier

_wfix_ctr = [0]


def _split_waits(nc, maxw=_MAXW):
    """Move excess sync-waits onto preceding same-engine nops."""
    n_split = 0
    for f in nc.m.functions:
        for b in f.blocks:
            lst = b.instructions
            i = 0
            while i < len(lst):
                inst = lst[i]
                si = getattr(inst, "sync_info", None)
                if si is not None:
                    waits = list(si.on_wait)
                    if len(waits) > maxw:
                        n_split += 1
                        inst.sync_info = mybir.SyncInfo(
                            on_wait=waits[:maxw], on_update=list(si.on_update)
                        )
                        rest = waits[maxw:]
                        for j in range(0, len(rest), maxw):
                            nop = mybir.InstNoOp(
                                name=f"WFIX-{_wfix_ctr[0]}", ins=[], outs=[]
                            )
                            _wfix_ctr[0] += 1
                            nop.engine = inst.engine
                            nop.sync_info = mybir.SyncInfo(
                                on_wait=rest[j : j + maxw], on_update=[]
                            )
                            lst.insert(i, nop)
                            i += 1
                i += 1
    return n_split


def _thin_updates(nc, engines=("EngineType.PE",)):
    """Drop engine-clock sem-incs at positions nobody waits on, and renumber
    the remaining wait thresholds to ranks within the kept set.

    Sound because incs fire in retirement (program) order: a wait for the
    original value k is satisfied exactly when the k-th incrementer retires;
    we keep that instruction's inc and rewrite the wait to its rank among
    kept incs. Only applied to sems whose incs all live in ONE basic block
    (so positional mapping is unambiguous)."""
    # thresholds per sem (across all blocks)
    thresholds = {}
    inc_engines = {}
    _bb_incs = {}
    for f in nc.m.functions:
        for b in f.blocks:
            for inst in b.instructions:
                si = getattr(inst, "sync_info", None)
                if si is None:
                    continue
                for w in si.on_wait:
                    if w.wait_value is not None and w.ant_name:
                        thresholds.setdefault(w.ant_name, set()).add(
                            w.wait_value
                        )
                for u in si.on_update:
                    if u.update_mode == "sem-inc" and u.ant_name:
                        inc_engines.setdefault(u.ant_name, set()).add(
                            str(inst.engine)
                        )
                        k = (u.ant_name, id(b))
                        _bb_incs[k] = _bb_incs.get(k, 0) + 1
    eligible = {
        s for s, engs in inc_engines.items()
        if engs == set(engines) and s in thresholds
    }
    # only thin inside each sem's main bb (loop skip/reset bbs keep incs)
    main_bb = {}
    for (s, bid), n in _bb_incs.items():
        if s in eligible and n > main_bb.get(s, (None, 0))[1]:
            main_bb[s] = (bid, n)
    remap = {}  # sem -> {old_value: new_value}
    n_removed = 0
    for f in nc.m.functions:
        for b in f.blocks:
            cum = {}
            kept = {}
            for inst in b.instructions:
                si = getattr(inst, "sync_info", None)
                if si is None or str(inst.engine) not in engines:
                    continue
                keep = []
                changed = False
                for u in si.on_update:
                    if (
                        u.update_mode == "sem-inc"
                        and u.update_value == 1
                        and u.ant_name in eligible
                        and not u.update_reg
                        and main_bb[u.ant_name][0] == id(b)
                    ):
                        s = u.ant_name
                        cum[s] = cum.get(s, 0) + 1
                        if cum[s] in thresholds[s]:
                            kept[s] = kept.get(s, 0) + 1
                            remap.setdefault(s, {})[cum[s]] = kept[s]
                            keep.append(u)
                        else:
                            n_removed += 1
                            changed = True
                    else:
                        keep.append(u)
                if changed:
                    inst.sync_info = mybir.SyncInfo(
                        on_wait=list(si.on_wait), on_update=keep
                    )
    # rewrite waits
    for f in nc.m.functions:
        for b in f.blocks:
            for inst in b.instructions:
                si = getattr(inst, "sync_info", None)
                if si is None:
                    continue
                for w in si.on_wait:
                    m = remap.get(w.ant_name)
                    if m and w.wait_value in m:
                        w.wait_value = m[w.wait_value]
    return n_removed


# ----------------------------------------------------------------------------
# Problem constants
# ----------------------------------------------------------------------------
B, T, L, O, H = 256, 512, 128, 64, 576
N_CORES = 8
BC = B // N_CORES  # 32 batch per core
NCH = 5  # hidden chunks: 128*4 + 64
CHW = [128, 128, 128, 128, 64]
F32 = mybir.dt.float32
AF = mybir.ActivationFunctionType
ALU = mybir.AluOpType


def build_nc(T_steps=T, cd=mybir.dt.bfloat16, loop_niter=None):
    """Build the per-core SPMD bass kernel (fully unrolled over T_steps).

    loop_niter: timing-only mode — wrap the T_steps-step body in a For_i
    hardware loop repeating it loop_niter times (outputs are garbage past
    the first pass; used to measure per-step HW time via wall deltas)."""
    nc = bass.Bass()
    TB = T_steps * BC

    # DRAM I/O (per core). Host packs these exactly in SBUF layout.
    d_xT = nc.dram_tensor("xT", [L, TB], cd, kind="ExternalInput")
    d_oh = nc.dram_tensor("OH", [2, TB], cd, kind="ExternalInput")
    d_w1 = nc.dram_tensor("W1s", [128, NCH * H], cd, kind="ExternalInput")
    d_w2 = nc.dram_tensor("W2s", [128, NCH * H], cd, kind="ExternalInput")
    d_i2h = nc.dram_tensor("I2Hs", [128, 512], cd, kind="ExternalInput")
    d_mv = nc.dram_tensor("MVs", [128, NCH * O], cd, kind="ExternalInput")
    d_mo = nc.dram_tensor("MOs", [128, NCH * O], cd, kind="ExternalInput")
    d_wtm = nc.dram_tensor("WTMs", [O, H], cd, kind="ExternalInput")
    d_thm = nc.dram_tensor("THM", [O, O], cd, kind="ExternalInput")
    d_id = nc.dram_tensor("ID64", [O, O], cd, kind="ExternalInput")
    d_bv = nc.dram_tensor("BV", [1, H], cd, kind="ExternalInput")
    d_bm = nc.dram_tensor("BM2", [2, H], cd, kind="ExternalInput")
    d_bp = nc.dram_tensor("BP", [2, O], cd, kind="ExternalInput")
    d_bo = nc.dram_tensor("BO", [1, O], cd, kind="ExternalInput")
    d_h0v = nc.dram_tensor("H0V", [128, NCH * BC], cd, kind="ExternalInput")
    d_h0m = nc.dram_tensor("H0M", [128, NCH * BC], cd, kind="ExternalInput")
    d_out = nc.dram_tensor("OUT", [O, TB], F32, kind="ExternalOutput")

    frees = []
    with tile.TileContext(nc) as tc:

        def mk(name, shape, dtype):
            t, fr = tc.tile(shape, dtype, name=name)
            frees.append(fr)
            return t

        xTs = mk("xTs", [L, TB], cd)
        oh = mk("oh", [2, TB], cd)
        w1 = mk("w1", [128, NCH * H], cd)
        w2 = mk("w2", [128, NCH * H], cd)
        i2h = mk("i2h", [128, 512], cd)
        mv = mk("mv", [128, NCH * O], cd)
        mo = mk("mo", [128, NCH * O], cd)
        wtm = mk("wtm", [O, H], cd)
        thm = mk("thm", [O, O], cd)
        id64 = mk("id64", [O, O], cd)
        bv = mk("bv", [1, H], cd)
        bm = mk("bm", [2, H], cd)
        bp = mk("bp", [2, O], cd)
        bo = mk("bo", [1, O], cd)
        hv = mk("hv", [128, NCH * BC], cd)
        hm = mk("hm", [128, NCH * BC], cd)
        ot = mk("ot", [O, BC], cd)
        om = mk("om", [O, BC], cd)
        outb = mk("outb", [O, TB], F32)

        # --- load everything (xT split along T for early compute start) ---
        NXCH = 8
        xch = TB // NXCH
        for i in range(NXCH):
            nc.sync.dma_start(
                out=xTs[:, i * xch : (i + 1) * xch],
                in_=d_xT[:, i * xch : (i + 1) * xch],
            )
        nc.sync.dma_start(out=oh, in_=d_oh[:, :])
        nc.sync.dma_start(out=w1, in_=d_w1[:, :])
        nc.sync.dma_start(out=w2, in_=d_w2[:, :])
        nc.sync.dma_start(out=i2h, in_=d_i2h[:, :])
        nc.sync.dma_start(out=mv, in_=d_mv[:, :])
        nc.sync.dma_start(out=mo, in_=d_mo[:, :])
        nc.sync.dma_start(out=wtm, in_=d_wtm[:, :])
        nc.sync.dma_start(out=thm, in_=d_thm[:, :])
        nc.sync.dma_start(out=id64, in_=d_id[:, :])
        nc.sync.dma_start(out=bv, in_=d_bv[:, :])
        nc.sync.dma_start(out=bm, in_=d_bm[:, :])
        nc.sync.dma_start(out=bp, in_=d_bp[:, :])
        nc.sync.dma_start(out=bo, in_=d_bo[:, :])
        nc.sync.dma_start(out=hv, in_=d_h0v[:, :])
        nc.sync.dma_start(out=hm, in_=d_h0m[:, :])

        _pp_cm = tc.tile_pool(name="psum", bufs=2, space="PSUM")
        pp = _pp_cm.__enter__()

        def mm(out_ap, lhsT, rhs, start, stop):
            nc.tensor.matmul(
                out_ap, lhsT, rhs, start=start, stop=stop, skip_group_check=True
            )

        def emit_group(mms):
            n = len(mms)
            for i, (out_ap, lhsT, rhs) in enumerate(mms):
                mm(out_ap, lhsT, rhs, i == 0, i == n - 1)

        # -- per-step sub-emitters (software-pipelined emission order) --
        state = {}

        def zv_mms(t, first):
            ts = slice(t * BC, (t + 1) * BC)
            zv = state["zv", t] = pp.tile([128, NCH * BC], F32, tag="zv", name=f"zv{t}")
            g = []
            for m in range(NCH):
                m0, mw = m * 128, CHW[m]
                oc = slice(m * BC, m * BC + BC)
                for k in range(NCH):
                    kw = CHW[k]
                    g.append((zv[0:mw, oc],
                              w1[0:kw, k * H + m0 : k * H + m0 + mw],
                              hv[0:kw, k * BC : k * BC + BC]))
                if m < 4:
                    g.append((zv[0:mw, oc], i2h[0:128, m0 : m0 + mw],
                              xTs[:, ts]))
                g.append((zv[0:mw, oc], bv[0:1, m0 : m0 + mw], oh[0:1, ts]))
            if not first:
                g.append((zv[0:64, 4 * BC : 5 * BC], id64[0:64, 0:64],
                          ot[0:64, :]))
            return g

        def act_hv(t):
            zv = state.pop(("zv", t))
            nc.scalar.activation(hv[0:128, 0 : 4 * BC], zv[0:128, 0 : 4 * BC],
                                 AF.Tanh)
            nc.scalar.activation(hv[0:64, 4 * BC : 5 * BC],
                                 zv[0:64, 4 * BC : 5 * BC], AF.Tanh)
            nc.vector.tensor_scalar_max(hv[0:128, 0 : 4 * BC],
                                        hv[0:128, 0 : 4 * BC], 0.0)
            nc.vector.tensor_scalar_max(hv[0:64, 4 * BC : 5 * BC],
                                        hv[0:64, 4 * BC : 5 * BC], 0.0)

        def emit_zmW2(t):
            ts = slice(t * BC, (t + 1) * BC)
            zm = state["zm", t] = pp.tile([128, NCH * BC], F32, tag="zm", name=f"zm{t}")
            zmfirst = True
            for m in range(NCH):
                m0, mw = m * 128, CHW[m]
                oc = slice(m * BC, m * BC + BC)
                for k in range(NCH):
                    kw = CHW[k]
                    mm(
                        zm[0:mw, oc],
                        w2[0:kw, k * H + m0 : k * H + m0 + mw],
                        hm[0:kw, k * BC : k * BC + BC],
                        zmfirst,
                        False,
                    )
                    zmfirst = False
                mm(zm[0:mw, oc], bm[0:2, m0 : m0 + mw], oh[0:2, ts], False,
                   False)

        def emit_p(t, first):
            ts = slice(t * BC, (t + 1) * BC)
            pt = state["pt", t] = pp.tile([O, BC], F32, tag="pt", name=f"pt{t}")
            pfirst = True
            for k in range(NCH):
                kw = CHW[k]
                mm(pt[0:64, :], mv[0:kw, k * O : k * O + O],
                   hv[0:kw, k * BC : k * BC + BC], pfirst, False)
                pfirst = False
            if not first:
                mm(pt[0:64, :], thm[0:64, 0:64], om[0:64, :], False, False)
            mm(pt[0:64, :], bp[0:2, 0:64], oh[0:2, ts], False, True)

        def act_ot(t):
            pt = state.pop(("pt", t))
            nc.scalar.activation(ot[0:64, :], pt[0:64, :], AF.Tanh)
            nc.vector.tensor_scalar_max(ot[0:64, :], ot[0:64, :], 0.0)

        def emit_zmWtm(t):
            zm = state[("zm", t)]
            for m in range(NCH):
                m0, mw = m * 128, CHW[m]
                oc = slice(m * BC, m * BC + BC)
                mm(zm[0:mw, oc], wtm[0:64, m0 : m0 + mw], ot[0:64, :], False,
                   m == NCH - 1)

        def act_hm(t):
            zm = state.pop(("zm", t))
            nc.scalar.activation(hm[0:128, 0 : 4 * BC], zm[0:128, 0 : 4 * BC],
                                 AF.Tanh)
            nc.scalar.activation(hm[0:64, 4 * BC : 5 * BC],
                                 zm[0:64, 4 * BC : 5 * BC], AF.Tanh)
            nc.vector.tensor_scalar_max(hm[0:128, 0 : 4 * BC],
                                        hm[0:128, 0 : 4 * BC], 0.0)
            nc.vector.tensor_scalar_max(hm[0:64, 4 * BC : 5 * BC],
                                        hm[0:64, 4 * BC : 5 * BC], 0.0)

        def emit_om(t):
            ts = slice(t * BC, (t + 1) * BC)
            zo = pp.tile([O, BC], F32, tag="zo", name=f"zo{t}")
            ofirst = True
            for k in range(NCH):
                kw = CHW[k]
                mm(zo[0:64, :], mo[0:kw, k * O : k * O + O],
                   hm[0:kw, k * BC : k * BC + BC], ofirst, False)
                ofirst = False
            mm(zo[0:64, :], bo[0:1, 0:64], oh[0:1, ts], False, True)
            nc.scalar.activation(outb[0:64, ts], zo[0:64, :], AF.Copy)
            nc.vector.tensor_copy(om[0:64, :], zo[0:64, :])

        ZV_HEAD = 6  # zv MMs emitted before the previous step's tail groups

        def pipeline(ts_list, first_t):
            """Emit steps with the serial tail of step t-1 interleaved into
            the matmul-heavy groups of step t so PE never starves."""
            for i, t in enumerate(ts_list):
                first = (t == first_t) and (i == 0)
                g = zv_mms(t, first)
                if i == 0:
                    emit_group(g)
                    act_hv(t)
                else:
                    tprev = ts_list[i - 1]
                    # zv(t) head — PE filler while ot(t-1) finishes
                    emit_group_partial(g, 0, ZV_HEAD, len(g))
                    emit_zmWtm(tprev)
                    act_hm(tprev)
                    emit_group_partial(g, ZV_HEAD, len(g), len(g))
                    act_hv(t)
                    emit_om(tprev)
                emit_zmW2(t)
                emit_p(t, first)
                act_ot(t)
            tlast = ts_list[-1]
            emit_zmWtm(tlast)
            act_hm(tlast)
            emit_om(tlast)

        def emit_group_partial(mms, lo, hi, n):
            for i in range(lo, hi):
                out_ap, lhsT, rhs = mms[i]
                mm(out_ap, lhsT, rhs, i == 0, i == n - 1)

        if loop_niter is None:
            pipeline(list(range(T_steps)), 0)
        else:
            nc.vector.memset(ot[:, :], 0.0)
            nc.vector.memset(om[:, :], 0.0)
            with tc.For_i(0, loop_niter, 1):
                pipeline(list(range(T_steps)), -1)

        # final store
        NOCH = 8
        och = TB // NOCH
        for i in range(NOCH):
            nc.sync.dma_start(
                out=d_out[:, i * och : (i + 1) * och],
                in_=outb[:, i * och : (i + 1) * och],
            )
        _pp_cm.__exit__(None, None, None)
        for fr in reversed(frees):
            fr()

    if loop_niter is None:
        # sem-inc thinning is validated for the straight-line kernel only
        # (For_i loop machinery keeps its own per-iteration clock resets)
        _thin_updates(nc)
    _split_waits(nc)
    return nc


# ----------------------------------------------------------------------------
# Host-side packing
# ----------------------------------------------------------------------------
def _np_dt(cd):
    if cd == mybir.dt.bfloat16:
        import ml_dtypes

        return ml_dtypes.bfloat16
    return np.float32


def pack_inputs(inputs, T_steps=T, cd=mybir.dt.bfloat16):
    npdt = _np_dt(cd)
    d = inputs

    def C(x):
        return np.ascontiguousarray(x.astype(np.float32)).astype(npdt)

    # shared (replicated) weights
    w1f = np.zeros((128, NCH * H), np.float32)
    w2f = np.zeros((128, NCH * H), np.float32)
    W1T = d["h2h_w"].T.astype(np.float32)  # [576, 576]
    W2T = d["h2hd_w"].T.astype(np.float32)
    for k in range(NCH):
        kw = CHW[k]
        w1f[0:kw, k * H : (k + 1) * H] = W1T[k * 128 : k * 128 + kw, :]
        w2f[0:kw, k * H : (k + 1) * H] = W2T[k * 128 : k * 128 + kw, :]
    Mv = (d["thal_w"][:, :64].astype(np.float32)
          @ d["h2o_w"].astype(np.float32)).T  # [576, 64]
    MoT = d["h2od_w"].T.astype(np.float32)  # [576, 64]
    mvf = np.zeros((128, NCH * O), np.float32)
    mof = np.zeros((128, NCH * O), np.float32)
    for k in range(NCH):
        kw = CHW[k]
        mvf[0:kw, k * O : (k + 1) * O] = Mv[k * 128 : k * 128 + kw, :]
        mof[0:kw, k * O : (k + 1) * O] = MoT[k * 128 : k * 128 + kw, :]
    bias_v = d["h2h_b"].astype(np.float32).copy()
    bias_v[:512] += d["i2h_b"].astype(np.float32)
    bias_p = (d["thal_b"].astype(np.float32)
              + d["thal_w"][:, :64].astype(np.float32)
              @ d["h2o_b"].astype(np.float32))
    holdw_p = d["thal_w"][:, 64].astype(np.float32)
    bias_m = d["i2hd_b"].astype(np.float32) + d["h2hd_b"].astype(np.float32)
    holdw_m = d["i2hd_w"][:, 64].astype(np.float32)

    shared = {
        "W1s": C(w1f),
        "W2s": C(w2f),
        "I2Hs": C(d["i2h_w"].T),
        "MVs": C(mvf),
        "MOs": C(mof),
        "WTMs": C(d["i2hd_w"][:, :64].T),
        "THM": C(d["thal_w"][:, 65:].T),
        "ID64": C(np.eye(64, dtype=np.float32)),
        "BV": C(bias_v[None, :]),
        "BM2": C(np.stack([bias_m, holdw_m])),
        "BP": C(np.stack([bias_p, holdw_p])),
        "BO": C(d["h2od_b"][None, :]),
    }

    data = d["data"].astype(np.float32)
    in_maps = []
    for c in range(N_CORES):
        s = slice(c * BC, (c + 1) * BC)
        dc = data[s, :T_steps, :]  # [BC, T, L]
        xT = np.ascontiguousarray(dc.transpose(2, 1, 0)).reshape(L, -1)
        ohrow = np.ones((2, T_steps * BC), np.float32)
        ohrow[1] = np.ascontiguousarray(dc[:, :, 127].T).reshape(-1)
        h0v = np.zeros((128, NCH * BC), np.float32)
        h0m = np.zeros((128, NCH * BC), np.float32)
        hv0 = d["h0_v"][s].astype(np.float32).T  # [576, BC]
        hm0 = d["h0_m"][s].astype(np.float32).T
        for k in range(NCH):
            kw = CHW[k]
            h0v[0:kw, k * BC : (k + 1) * BC] = hv0[k * 128 : k * 128 + kw, :]
            h0m[0:kw, k * BC : (k + 1) * BC] = hm0[k * 128 : k * 128 + kw, :]
        m = dict(shared)
        m["xT"] = xT.astype(_np_dt(cd))
        m["OH"] = ohrow.astype(_np_dt(cd))
        m["H0V"] = h0v.astype(_np_dt(cd))
        m["H0M"] = h0m.astype(_np_dt(cd))
        in_maps.append(m)
    return in_maps


def run(inputs, T_steps=T, cd=mybir.dt.bfloat16, nc=None):
    if nc is None:
        nc = build_nc(T_steps, cd)
    in_maps = pack_inputs(inputs, T_steps, cd)
    res = run_bass_kernel_spmd(nc, in_maps, core_ids=list(range(N_CORES)))
    out = np.zeros((B, T_steps, O), np.float32)
    for c in range(N_CORES):
        o = res.results[c]["OUT"]  # [O, T*BC]
        out[c * BC : (c + 1) * BC] = (
            o.reshape(O, T_steps, BC).transpose(2, 1, 0)
        )
    return out


def kernel(**inputs):
    return run(inputs)


if __name__ == "__main__":
    pass


# revision 22
# speedup vs baseline: 1453.1167x; 1.1965x over previous
"""Trainium2 Bass kernel for nn_MilliesRNN (B=256, T=512, L=128, O=64, H=576).

Strategy:
- Data-parallel over batch: 8 cores x 32 sequences each; all weights replicated.
- On each core the recurrent state is kept transposed ([hidden, batch] =
  [576->5 chunks of 128, 32]) so every matmul uses the weights as the
  stationary operand and the state as the 32-wide moving operand; states stay
  SBUF/PSUM-resident for all 512 steps.
- Algebraic refactor (validated to 2.5e-07 vs reference in fp32):
    z_v  = W1^T h_v + i2h^T x_t + bias_v (+ prev out_t on dims 512:576)
    h_v  = relu(tanh(z_v))
    p    = Mv^T h_v + thal_m^T out_m + bias_p + hold_t * holdw_p
    out_t= relu(tanh(p))           with Mv = (thal_w[:, :64] @ h2o_w)^T
    z_m  = W2^T h_m + Wtm^T out_t + bias_m + hold_t * holdw_m
    h_m  = relu(tanh(z_m))
    out_m= Mo^T h_m + h2od_b       (also the per-step output)
  Biases/hold enter as K=1/K=2 matmuls against a host-precomputed
  [ones; hold] stream.
- Compute dtype bf16 (weights + states; PSUM accumulation fp32, output fp32):
  measured 2.6e-03 rel L2 end-to-end in a bit-accurate numpy simulation.
"""

import numpy as np

import concourse.bass as bass
import concourse.mybir as mybir
import concourse.tile as tile
from concourse.bass_utils import run_bass_kernel_spmd
from concourse.vector_clock import ScopedClock

# ----------------------------------------------------------------------------
# Workarounds: this walrus build only supports ONE sync-wait per instruction.
# ----------------------------------------------------------------------------
_MAXW = 1


def _patched_drain_and_barrier(self, tick_clock, wait_clock):
    nc = self.nc
    drain_inst = nc.sync.drain()
    wait_clock.add_sem_waits(
        drain_inst.ins, ScopedClock({None: tick_clock.global_clock})
    )
    waits = list(drain_inst.ins.sync_info.on_wait)
    if len(waits) > _MAXW:
        drain_inst.ins.sync_info = mybir.SyncInfo(
            on_wait=waits[:_MAXW], on_update=[]
        )
        rest = waits[_MAXW:]
        for i in range(0, len(rest), _MAXW):
            nop = nc.sync.nop(nofuse=True)
            nop.ins.sync_info = mybir.SyncInfo(
                on_wait=rest[i : i + _MAXW], on_update=[]
            )
    nc.all_engine_barrier()
    assert self.sems is not None
    popped = nc._tile_sem_poison_stack.pop()
    assert popped is self._sem_poison
    nc.clear_and_free_semaphores(list(self.sems.allocated().values()))
    nc.all_engine_barrier()


tile.TileContext._drain_and_barrier = _patched_drain_and_barrier

_wfix_ctr = [0]


def _split_waits(nc, maxw=_MAXW):
    """Move excess sync-waits onto preceding same-engine nops."""
    n_split = 0
    for f in nc.m.functions:
        for b in f.blocks:
            lst = b.instructions
            i = 0
            while i < len(lst):
                inst = lst[i]
                si = getattr(inst, "sync_info", None)
                if si is not None:
                    waits = list(si.on_wait)
                    if len(waits) > maxw:
                        n_split += 1
                        inst.sync_info = mybir.SyncInfo(
                            on_wait=waits[:maxw], on_update=list(si.on_update)
                        )
                        rest = waits[maxw:]
                        for j in range(0, len(rest), maxw):
                            nop = mybir.InstNoOp(
                                name=f"WFIX-{_wfix_ctr[0]}", ins=[], outs=[]
                            )
                            _wfix_ctr[0] += 1
                            nop.engine = inst.engine
                            nop.sync_info = mybir.SyncInfo(
                                on_wait=rest[j : j + maxw], on_update=[]
                            )
                            lst.insert(i, nop)
                            i += 1
                i += 1
    return n_split


def _thin_updates(nc, engines=("EngineType.PE",)):
    """Drop engine-clock sem-incs at positions nobody waits on, and renumber
    the remaining wait thresholds to ranks within the kept set.

    Sound because incs fire in retirement (program) order: a wait for the
    original value k is satisfied exactly when the k-th incrementer retires;
    we keep that instruction's inc and rewrite the wait to its rank among
    kept incs. Only applied to sems whose incs all live in ONE basic block
    (so positional mapping is unambiguous)."""
    # thresholds per sem (across all blocks)
    thresholds = {}
    inc_engines = {}
    _bb_incs = {}
    for f in nc.m.functions:
        for b in f.blocks:
            for inst in b.instructions:
                si = getattr(inst, "sync_info", None)
                if si is None:
                    continue
                for w in si.on_wait:
                    if w.wait_value is not None and w.ant_name:
                        thresholds.setdefault(w.ant_name, set()).add(
                            w.wait_value
                        )
                for u in si.on_update:
                    if u.update_mode == "sem-inc" and u.ant_name:
                        inc_engines.setdefault(u.ant_name, set()).add(
                            str(inst.engine)
                        )
                        k = (u.ant_name, id(b))
                        _bb_incs[k] = _bb_incs.get(k, 0) + 1
    eligible = {
        s for s, engs in inc_engines.items()
        if engs == set(engines) and s in thresholds
    }
    # only thin inside each sem's main bb (loop skip/reset bbs keep incs)
    main_bb = {}
    for (s, bid), n in _bb_incs.items():
        if s in eligible and n > main_bb.get(s, (None, 0))[1]:
            main_bb[s] = (bid, n)
    remap = {}  # sem -> {old_value: new_value}
    n_removed = 0
    for f in nc.m.functions:
        for b in f.blocks:
            cum = {}
            kept = {}
            for inst in b.instructions:
                si = getattr(inst, "sync_info", None)
                if si is None or str(inst.engine) not in engines:
                    continue
                keep = []
                changed = False
                for u in si.on_update:
                    if (
                        u.update_mode == "sem-inc"
                        and u.update_value == 1
                        and u.ant_name in eligible
                        and not u.update_reg
                        and main_bb[u.ant_name][0] == id(b)
                    ):
                        s = u.ant_name
                        cum[s] = cum.get(s, 0) + 1
                        if cum[s] in thresholds[s]:
                            kept[s] = kept.get(s, 0) + 1
                            remap.setdefault(s, {})[cum[s]] = kept[s]
                            keep.append(u)
                        else:
                            n_removed += 1
                            changed = True
                    else:
                        keep.append(u)
                if changed:
                    inst.sync_info = mybir.SyncInfo(
                        on_wait=list(si.on_wait), on_update=keep
                    )
    # rewrite waits
    for f in nc.m.functions:
        for b in f.blocks:
            for inst in b.instructions:
                si = getattr(inst, "sync_info", None)
                if si is None:
                    continue
                for w in si.on_wait:
                    m = remap.get(w.ant_name)
                    if m and w.wait_value in m:
                        w.wait_value = m[w.wait_value]
    return n_removed


# ----------------------------------------------------------------------------
# Problem constants
# ----------------------------------------------------------------------------
B, T, L, O, H = 256, 512, 128, 64, 576
N_CORES = 8
BC = B // N_CORES  # 32 batch per core
NCH = 5  # hidden chunks: 128*4 + 64
CHW = [128, 128, 128, 128, 64]
F32 = mybir.dt.float32
AF = mybir.ActivationFunctionType
ALU = mybir.AluOpType


def build_nc(T_steps=T, cd=mybir.dt.bfloat16, loop_niter=None):
    """Build the per-core SPMD bass kernel (fully unrolled over T_steps).

    loop_niter: timing-only mode — wrap the T_steps-step body in a For_i
    hardware loop repeating it loop_niter times (outputs are garbage past
    the first pass; used to measure per-step HW time via wall deltas)."""
    nc = bass.Bass()
    TB = T_steps * BC

    # DRAM I/O (per core). Host packs these exactly in SBUF layout.
    d_xT = nc.dram_tensor("xT", [L, TB], cd, kind="ExternalInput")
    d_oh = nc.dram_tensor("OH", [2, TB], cd, kind="ExternalInput")
    d_w1 = nc.dram_tensor("W1s", [128, NCH * H], cd, kind="ExternalInput")
    d_w2 = nc.dram_tensor("W2s", [128, NCH * H], cd, kind="ExternalInput")
    d_i2h = nc.dram_tensor("I2Hs", [128, 512], cd, kind="ExternalInput")
    d_mv = nc.dram_tensor("MVs", [128, NCH * O], cd, kind="ExternalInput")
    d_mo = nc.dram_tensor("MOs", [128, NCH * O], cd, kind="ExternalInput")
    d_wtm = nc.dram_tensor("WTMs", [O, H], cd, kind="ExternalInput")
    d_thm = nc.dram_tensor("THM", [O, O], cd, kind="ExternalInput")
    d_id = nc.dram_tensor("ID64", [O, O], cd, kind="ExternalInput")
    d_bm = nc.dram_tensor("BM2", [2, H], cd, kind="ExternalInput")
    d_bp = nc.dram_tensor("BP", [2, O], cd, kind="ExternalInput")
    d_h0v = nc.dram_tensor("H0V", [128, NCH * BC], cd, kind="ExternalInput")
    d_h0m = nc.dram_tensor("H0M", [128, NCH * BC], cd, kind="ExternalInput")
    d_out = nc.dram_tensor("OUT", [O, TB], F32, kind="ExternalOutput")

    frees = []
    with tile.TileContext(nc) as tc:

        def mk(name, shape, dtype):
            t, fr = tc.tile(shape, dtype, name=name)
            frees.append(fr)
            return t

        xTs = mk("xTs", [L, TB], cd)
        oh = mk("oh", [2, TB], cd)
        w1 = mk("w1", [128, NCH * H], cd)
        w2 = mk("w2", [128, NCH * H], cd)
        i2h = mk("i2h", [128, 512], cd)
        mv = mk("mv", [128, NCH * O], cd)
        mo = mk("mo", [128, NCH * O], cd)
        wtm = mk("wtm", [O, H], cd)
        thm = mk("thm", [O, O], cd)
        id64 = mk("id64", [O, O], cd)
        bm = mk("bm", [2, H], cd)
        bp = mk("bp", [2, O], cd)
        hv = mk("hv", [128, NCH * BC], cd)
        hm = mk("hm", [128, NCH * BC], cd)
        ot = mk("ot", [O, BC], cd)
        om = mk("om", [O, BC], cd)
        outb = mk("outb", [O, TB], F32)

        # --- load everything (xT split along T for early compute start) ---
        NXCH = 8
        xch = TB // NXCH
        for i in range(NXCH):
            nc.sync.dma_start(
                out=xTs[:, i * xch : (i + 1) * xch],
                in_=d_xT[:, i * xch : (i + 1) * xch],
            )
        nc.sync.dma_start(out=oh, in_=d_oh[:, :])
        nc.sync.dma_start(out=w1, in_=d_w1[:, :])
        nc.sync.dma_start(out=w2, in_=d_w2[:, :])
        nc.sync.dma_start(out=i2h, in_=d_i2h[:, :])
        nc.sync.dma_start(out=mv, in_=d_mv[:, :])
        nc.sync.dma_start(out=mo, in_=d_mo[:, :])
        nc.sync.dma_start(out=wtm, in_=d_wtm[:, :])
        nc.sync.dma_start(out=thm, in_=d_thm[:, :])
        nc.sync.dma_start(out=id64, in_=d_id[:, :])
        nc.sync.dma_start(out=bm, in_=d_bm[:, :])
        nc.sync.dma_start(out=bp, in_=d_bp[:, :])
        nc.sync.dma_start(out=hv, in_=d_h0v[:, :])
        nc.sync.dma_start(out=hm, in_=d_h0m[:, :])

        _pp_cm = tc.tile_pool(name="psum", bufs=2, space="PSUM")
        pp = _pp_cm.__enter__()

        def mm(out_ap, lhsT, rhs, start, stop):
            nc.tensor.matmul(
                out_ap, lhsT, rhs, start=start, stop=stop, skip_group_check=True
            )

        def emit_group(mms):
            n = len(mms)
            for i, (out_ap, lhsT, rhs) in enumerate(mms):
                mm(out_ap, lhsT, rhs, i == 0, i == n - 1)

        # -- per-step sub-emitters (software-pipelined emission order) --
        state = {}

        def zv_mms(t, first):
            ts = slice(t * BC, (t + 1) * BC)
            zv = state["zv", t] = pp.tile([128, NCH * BC], F32, tag="zv", name=f"zv{t}")
            g = []
            for m in range(NCH):
                m0, mw = m * 128, CHW[m]
                oc = slice(m * BC, m * BC + BC)
                for k in range(NCH):
                    # k==4: K=65 — row 64 of the state's chunk-4 block is a
                    # static ones row, row 64 of w1's k4 block is bias_v
                    kw = CHW[k] if k < 4 else 65
                    g.append((zv[0:mw, oc],
                              w1[0:kw, k * H + m0 : k * H + m0 + mw],
                              hv[0:kw, k * BC : k * BC + BC]))
                if m < 4:
                    g.append((zv[0:mw, oc], i2h[0:128, m0 : m0 + mw],
                              xTs[:, ts]))
            if not first:
                g.append((zv[0:64, 4 * BC : 5 * BC], id64[0:64, 0:64],
                          ot[0:64, :]))
            return g

        def act_hv(t):
            zv = state.pop(("zv", t))
            nc.scalar.activation(hv[0:128, 0 : 4 * BC], zv[0:128, 0 : 4 * BC],
                                 AF.Tanh)
            nc.scalar.activation(hv[0:64, 4 * BC : 5 * BC],
                                 zv[0:64, 4 * BC : 5 * BC], AF.Tanh)
            nc.vector.tensor_scalar_max(hv[0:128, 0 : 4 * BC],
                                        hv[0:128, 0 : 4 * BC], 0.0)
            nc.vector.tensor_scalar_max(hv[0:64, 4 * BC : 5 * BC],
                                        hv[0:64, 4 * BC : 5 * BC], 0.0)

        def emit_zmW2(t):
            ts = slice(t * BC, (t + 1) * BC)
            zm = state["zm", t] = pp.tile([128, NCH * BC], F32, tag="zm", name=f"zm{t}")
            zmfirst = True
            for m in range(NCH):
                m0, mw = m * 128, CHW[m]
                oc = slice(m * BC, m * BC + BC)
                for k in range(NCH):
                    kw = CHW[k]
                    mm(
                        zm[0:mw, oc],
                        w2[0:kw, k * H + m0 : k * H + m0 + mw],
                        hm[0:kw, k * BC : k * BC + BC],
                        zmfirst,
                        False,
                    )
                    zmfirst = False
                mm(zm[0:mw, oc], bm[0:2, m0 : m0 + mw], oh[0:2, ts], False,
                   False)

        def emit_p(t, first):
            ts = slice(t * BC, (t + 1) * BC)
            pt = state["pt", t] = pp.tile([O, BC], F32, tag="pt", name=f"pt{t}")
            pfirst = True
            for k in range(NCH):
                kw = CHW[k]
                mm(pt[0:64, :], mv[0:kw, k * O : k * O + O],
                   hv[0:kw, k * BC : k * BC + BC], pfirst, False)
                pfirst = False
            if not first:
                mm(pt[0:64, :], thm[0:64, 0:64], om[0:64, :], False, False)
            mm(pt[0:64, :], bp[0:2, 0:64], oh[0:2, ts], False, True)

        def act_ot(t):
            pt = state.pop(("pt", t))
            nc.scalar.activation(ot[0:64, :], pt[0:64, :], AF.Tanh)
            nc.vector.tensor_scalar_max(ot[0:64, :], ot[0:64, :], 0.0)

        def emit_zmWtm(t):
            zm = state[("zm", t)]
            for m in range(NCH):
                m0, mw = m * 128, CHW[m]
                oc = slice(m * BC, m * BC + BC)
                mm(zm[0:mw, oc], wtm[0:64, m0 : m0 + mw], ot[0:64, :], False,
                   m == NCH - 1)

        def act_hm(t):
            zm = state.pop(("zm", t))
            nc.scalar.activation(hm[0:128, 0 : 4 * BC], zm[0:128, 0 : 4 * BC],
                                 AF.Tanh)
            nc.scalar.activation(hm[0:64, 4 * BC : 5 * BC],
                                 zm[0:64, 4 * BC : 5 * BC], AF.Tanh)
            nc.vector.tensor_scalar_max(hm[0:128, 0 : 4 * BC],
                                        hm[0:128, 0 : 4 * BC], 0.0)
            nc.vector.tensor_scalar_max(hm[0:64, 4 * BC : 5 * BC],
                                        hm[0:64, 4 * BC : 5 * BC], 0.0)

        def emit_om(t):
            ts = slice(t * BC, (t + 1) * BC)
            zo = pp.tile([O, BC], F32, tag="zo", name=f"zo{t}")
            for k in range(NCH):
                # k==4: K=65 ones-row carries h2od_b (row 64 of mo's k4 block)
                kw = CHW[k] if k < 4 else 65
                mm(zo[0:64, :], mo[0:kw, k * O : k * O + O],
                   hm[0:kw, k * BC : k * BC + BC], k == 0, k == NCH - 1)
            nc.scalar.activation(outb[0:64, ts], zo[0:64, :], AF.Copy)
            nc.vector.tensor_copy(om[0:64, :], zo[0:64, :])

        ZV_HEAD = 6  # zv MMs emitted before the previous step's tail groups

        def pipeline(ts_list, first_t):
            """Emit steps with the serial tail of step t-1 interleaved into
            the matmul-heavy groups of step t so PE never starves."""
            for i, t in enumerate(ts_list):
                first = (t == first_t) and (i == 0)
                g = zv_mms(t, first)
                if i == 0:
                    emit_group(g)
                    act_hv(t)
                else:
                    tprev = ts_list[i - 1]
                    # zv(t) head — PE filler while ot(t-1) finishes
                    emit_group_partial(g, 0, ZV_HEAD, len(g))
                    emit_zmWtm(tprev)
                    act_hm(tprev)
                    emit_group_partial(g, ZV_HEAD, len(g), len(g))
                    act_hv(t)
                    emit_om(tprev)
                emit_zmW2(t)
                emit_p(t, first)
                act_ot(t)
            tlast = ts_list[-1]
            emit_zmWtm(tlast)
            act_hm(tlast)
            emit_om(tlast)

        def emit_group_partial(mms, lo, hi, n):
            for i in range(lo, hi):
                out_ap, lhsT, rhs = mms[i]
                mm(out_ap, lhsT, rhs, i == 0, i == n - 1)

        if loop_niter is None:
            pipeline(list(range(T_steps)), 0)
        else:
            nc.vector.memset(ot[:, :], 0.0)
            nc.vector.memset(om[:, :], 0.0)
            with tc.For_i(0, loop_niter, 1):
                pipeline(list(range(T_steps)), -1)

        # final store
        NOCH = 8
        och = TB // NOCH
        for i in range(NOCH):
            nc.sync.dma_start(
                out=d_out[:, i * och : (i + 1) * och],
                in_=outb[:, i * och : (i + 1) * och],
            )
        _pp_cm.__exit__(None, None, None)
        for fr in reversed(frees):
            fr()

    if loop_niter is None:
        # sem-inc thinning is validated for the straight-line kernel only
        # (For_i loop machinery keeps its own per-iteration clock resets)
        _thin_updates(nc)
    _split_waits(nc)
    return nc


# ----------------------------------------------------------------------------
# Host-side packing
# ----------------------------------------------------------------------------
def _np_dt(cd):
    if cd == mybir.dt.bfloat16:
        import ml_dtypes

        return ml_dtypes.bfloat16
    return np.float32


def pack_inputs(inputs, T_steps=T, cd=mybir.dt.bfloat16):
    npdt = _np_dt(cd)
    d = inputs

    def C(x):
        return np.ascontiguousarray(x.astype(np.float32)).astype(npdt)

    # shared (replicated) weights
    w1f = np.zeros((128, NCH * H), np.float32)
    w2f = np.zeros((128, NCH * H), np.float32)
    W1T = d["h2h_w"].T.astype(np.float32)  # [576, 576]
    W2T = d["h2hd_w"].T.astype(np.float32)
    for k in range(NCH):
        kw = CHW[k]
        w1f[0:kw, k * H : (k + 1) * H] = W1T[k * 128 : k * 128 + kw, :]
        w2f[0:kw, k * H : (k + 1) * H] = W2T[k * 128 : k * 128 + kw, :]
    Mv = (d["thal_w"][:, :64].astype(np.float32)
          @ d["h2o_w"].astype(np.float32)).T  # [576, 64]
    MoT = d["h2od_w"].T.astype(np.float32)  # [576, 64]
    mvf = np.zeros((128, NCH * O), np.float32)
    mof = np.zeros((128, NCH * O), np.float32)
    for k in range(NCH):
        kw = CHW[k]
        mvf[0:kw, k * O : (k + 1) * O] = Mv[k * 128 : k * 128 + kw, :]
        mof[0:kw, k * O : (k + 1) * O] = MoT[k * 128 : k * 128 + kw, :]
    bias_v = d["h2h_b"].astype(np.float32).copy()
    bias_v[:512] += d["i2h_b"].astype(np.float32)
    w1f[64, 4 * H : 5 * H] = bias_v
    mof[64, 4 * O : 5 * O] = d["h2od_b"].astype(np.float32)
    bias_p = (d["thal_b"].astype(np.float32)
              + d["thal_w"][:, :64].astype(np.float32)
              @ d["h2o_b"].astype(np.float32))
    holdw_p = d["thal_w"][:, 64].astype(np.float32)
    bias_m = d["i2hd_b"].astype(np.float32) + d["h2hd_b"].astype(np.float32)
    holdw_m = d["i2hd_w"][:, 64].astype(np.float32)

    shared = {
        "W1s": C(w1f),
        "W2s": C(w2f),
        "I2Hs": C(d["i2h_w"].T),
        "MVs": C(mvf),
        "MOs": C(mof),
        "WTMs": C(d["i2hd_w"][:, :64].T),
        "THM": C(d["thal_w"][:, 65:].T),
        "ID64": C(np.eye(64, dtype=np.float32)),
        "BM2": C(np.stack([bias_m, holdw_m])),
        "BP": C(np.stack([bias_p, holdw_p])),
    }

    data = d["data"].astype(np.float32)
    in_maps = []
    for c in range(N_CORES):
        s = slice(c * BC, (c + 1) * BC)
        dc = data[s, :T_steps, :]  # [BC, T, L]
        xT = np.ascontiguousarray(dc.transpose(2, 1, 0)).reshape(L, -1)
        ohrow = np.ones((2, T_steps * BC), np.float32)
        ohrow[1] = np.ascontiguousarray(dc[:, :, 127].T).reshape(-1)
        h0v = np.zeros((128, NCH * BC), np.float32)
        h0m = np.zeros((128, NCH * BC), np.float32)
        hv0 = d["h0_v"][s].astype(np.float32).T  # [576, BC]
        hm0 = d["h0_m"][s].astype(np.float32).T
        for k in range(NCH):
            kw = CHW[k]
            h0v[0:kw, k * BC : (k + 1) * BC] = hv0[k * 128 : k * 128 + kw, :]
            h0m[0:kw, k * BC : (k + 1) * BC] = hm0[k * 128 : k * 128 + kw, :]
        h0v[64, 4 * BC : 5 * BC] = 1.0  # static ones row (bias via K=65 MMs)
        h0m[64, 4 * BC : 5 * BC] = 1.0
        m = dict(shared)
        m["xT"] = xT.astype(_np_dt(cd))
        m["OH"] = ohrow.astype(_np_dt(cd))
        m["H0V"] = h0v.astype(_np_dt(cd))
        m["H0M"] = h0m.astype(_np_dt(cd))
        in_maps.append(m)
    return in_maps


def run(inputs, T_steps=T, cd=mybir.dt.bfloat16, nc=None):
    if nc is None:
        nc = build_nc(T_steps, cd)
    in_maps = pack_inputs(inputs, T_steps, cd)
    res = run_bass_kernel_spmd(nc, in_maps, core_ids=list(range(N_CORES)))
    out = np.zeros((B, T_steps, O), np.float32)
    for c in range(N_CORES):
        o = res.results[c]["OUT"]  # [O, T*BC]
        out[c * BC : (c + 1) * BC] = (
            o.reshape(O, T_steps, BC).transpose(2, 1, 0)
        )
    return out


def kernel(**inputs):
    return run(inputs)


if __name__ == "__main__":
    pass
